# revision 1
# baseline (speedup 1.0000x reference)
"""Trainium2 Bass kernel for nn_Attention_45148696216373.

8-core data-parallel over tokens (B*S = 131072 -> 16384/core); x is
pre-transposed on the host to [128c, tokens] so channel sits on SBUF
partitions for the PE matmul. All scalar constants (1/sqrt(D), the
1/H of the head-mean) and the output projection Wo are folded into a
single 960-wide fused projection [Q(256)|msum(64)|K0(128)|VW(512)]
run at full PE rate via float32r (1 cyc/row at N>=256). Bias rides as
a K=1 ones-row matmul accumulated into the same PSUM banks. The
per-token bilinear part (attention scores + output combine) runs on
the Vector engine in fp16 at 2x perf mode, staged PSUM->SBUF by the
Scalar engine; per-op overhead amortized by spanning 8 token-tiles
per instruction. Modeled: ~271us/core, DVE-bound (90%), vs ~71us DMA
roofline; rel err ~7e-4.

Math (per token t, all ops independent across tokens):
  q_st = x @ Wq^T + bq   -> [D,H] raw-reshaped to [H,D]  (index scramble)
  k_st, v_st similarly -> [KV,D]
  msum = sum_h q[h,:]                      (mean*4; /4 folded into Wvo)
  km   = k0 * msum                         (k scaled by q-mean)
  attn[h,k] = sum_d q[h,d]*km[k,d]         (/sqrt(D) folded into Wvo)
  out[h,:]  = sum_k attn[h,k]*v[k,:]
  y = reshape(out)[2 rows of 128] @ Wo^T

Device formulation: fold Wo into the V projection on the host:
  VW[t,(r,k),o] = (1/32) * sum_d v[t,k*64+d] * Wo[o, r*64+d]
  Y[t, j*128+o] = sum_{r,k} attn_raw[t,2j+r,k] * VW[t,(2r+k)*128+o]
One 960-wide fused projection per token: [Q(256) | msum(64) | K0(128) | VW(512)].
"""

import os

# The Bass SPMD path needs the axon trn2 PJRT backend; a cpu pin (e.g. from a
# reference-only harness env) would hide the 8 NeuronCores from jax.devices().
if os.environ.get("JAX_PLATFORMS", "").strip().lower() == "cpu":
    os.environ.pop("JAX_PLATFORMS")

import numpy as np

B, S, DIM = 16, 8192, 128
H, KV, D = 4, 2, 64
T = B * S                 # 131072 tokens
NCORES = 8
TPC = T // NCORES         # 16384 tokens per core
TT = 128                  # tokens per tile (partition dim)
NT = TPC // TT            # 128 tiles per core

NQ = H * D                # 256
NM = D                    # 64
NK = KV * D               # 128
NV = 4 * DIM              # 512 (VW block: q=(2r+k) blocks of 128)
NPROJ = NQ + NM + NK + NV  # 960
OQ, OM, OK, OV = 0, NQ, NQ + NM, NQ + NM + NK

_COMPILED = None


def _fold_weights(Wq, bq, Wk, bk, Wv, bv, Wo):
    """Build W_all [128, 960] and bias_all [960] (fp32)."""
    j = np.arange(NQ)
    Wq_p = Wq[j % H, j // H, :]            # [256, 128]
    bq_p = bq[j % H, j // H]               # [256]
    jk = np.arange(NK)
    Wk_p = Wk[jk % KV, jk // KV, :]        # [128, 128]
    bk_p = bk[jk % KV, jk // KV]
    Wv_p = Wv[jk % KV, jk // KV, :]        # [128, 128]
    bv_p = bv[jk % KV, jk // KV]

    # msum block: col d = sum_h Wq_p[h*64+d]
    Wm = Wq_p.reshape(H, D, DIM).sum(axis=0)     # [64, 128]
    bm = bq_p.reshape(H, D).sum(axis=0)          # [64]

    # VW block: row (q=2r+k)*128+o = (1/32) sum_d Wv_p[k*64+d,:]*Wo[o, r*64+d]
    Wvo = np.zeros((4, DIM, DIM), dtype=np.float64)
    bvo = np.zeros((4, DIM), dtype=np.float64)
    scale = 1.0 / 32.0
    for r in range(2):
        for k in range(2):
            q = 2 * r + k
            # [o, c] = sum_d Wo[o, r*64+d] * Wv_p[k*64+d, c]
            Wvo[q] = scale * (Wo[:, r * D:(r + 1) * D] @ Wv_p[k * D:(k + 1) * D, :])
            bvo[q] = scale * (Wo[:, r * D:(r + 1) * D] @ bv_p[k * D:(k + 1) * D])

    # VW block stored [o-major, q-minor]: col = OV + o*4 + q  (enables
    # innermost-q step-1 access in the Y-combine for DVE 2x mode)
    Wvo_oq = Wvo.transpose(1, 0, 2).reshape(4 * DIM, DIM)
    bvo_oq = bvo.T.reshape(4 * DIM)
    W_all = np.concatenate(
        [Wq_p, Wm, Wk_p, Wvo_oq], axis=0
    ).astype(np.float32)                               # [960, 128]
    b_all = np.concatenate(
        [bq_p, bm, bk_p, bvo_oq]
    ).astype(np.float32)                               # [960]
    return W_all.T.copy(), b_all                       # [128, 960], [960]


def _numpy_forward(x2d, W_all, b_all):
    """Host re-implementation of the device math (for validation)."""
    proj = x2d @ W_all + b_all                         # [t, 960]
    Q = proj[:, OQ:OQ + NQ].reshape(-1, H, D)
    msum = proj[:, OM:OM + NM]
    K0 = proj[:, OK:OK + NK].reshape(-1, KV, D)
    VW = proj[:, OV:OV + NV].reshape(-1, DIM, 4)   # [t, o, q]
    km = K0 * msum[:, None, :]
    attn = np.einsum("thd,tkd->thk", Q, km)            # [t, 4, 2]
    a = attn.reshape(-1, 2, 4)                         # [t, j, q=(2r+k)]
    # Y[t, j, o] = sum_q a[t,j,q] * VW[t, o, q]
    Y = np.einsum("tjq,toq->tjo", a, VW)
    return Y.reshape(-1, 2 * DIM)                      # [t, 256]


def _build_program():
    import concourse.bass as bass
    import concourse.tile as tile
    from concourse import bacc, mybir

    f32 = mybir.dt.float32
    f32r = mybir.dt.float32r
    bf16 = mybir.dt.float16  # fp16: same 2x DVE modes as bf16, 4x less rounding error

    nc = bacc.Bacc(
        "TRN2",
        target_bir_lowering=False,
        debug=False,
        enable_asserts=False,
        num_devices=NCORES,
    )

    xT_d = nc.dram_tensor("xT", [DIM, TPC], f32r, kind="ExternalInput").ap()
    w_d = nc.dram_tensor("wall", [DIM, NPROJ], f32r, kind="ExternalInput").ap()
    b_d = nc.dram_tensor("ball", [1, NPROJ], f32r, kind="ExternalInput").ap()
    one_d = nc.dram_tensor("ones", [1, TT], f32r, kind="ExternalInput").ap()
    y_d = nc.dram_tensor("y", [TPC, 2 * DIM], f32, kind="ExternalOutput").ap()

    with tile.TileContext(nc) as tc:
        with (
            tc.tile_pool(name="const", bufs=1) as cpool,
            tc.tile_pool(name="xin", bufs=3) as xpool,
            tc.tile_pool(name="psum", bufs=2, space="PSUM") as ppool,
            tc.tile_pool(name="work", bufs=3) as wpool,
            tc.tile_pool(name="yout", bufs=2) as ypool,
        ):
            w_sb = cpool.tile([DIM, NPROJ], f32r)
            nc.sync.dma_start(w_sb[:], w_d[:, :])
            b_sb = cpool.tile([1, NPROJ], f32r)
            nc.sync.dma_start(b_sb[:], b_d[:, :])
            one_sb = cpool.tile([1, TT], f32r)
            nc.sync.dma_start(one_sb[:], one_d[:, :])

            G = 2   # PSUM group (2 x [TT,2,1024]f32 tiles double-buffered = 16KB)
            GS = 8  # SBUF group: DVE/ACT ops span 8 tiles to amortize op overhead
            for g in range(NT // GS):
                xt = xpool.tile([DIM, GS * TT], f32r)
                nc.sync.dma_start(xt[:], xT_d[:, g * GS * TT:(g + 1) * GS * TT])

                stg = wpool.tile([TT, GS, 960], bf16)
                for sub in range(GS // G):
                    pp = ppool.tile([TT, G, 1024], f32, name=f"pp{sub}", tag="pp")
                    for v in range(G):
                        u = sub * G + v
                        for lo, hi in ((0, 512), (512, NPROJ)):
                            nc.tensor.matmul(
                                out=pp[:, v, lo:hi],
                                lhsT=one_sb[:, :],
                                rhs=b_sb[:, lo:hi],
                                start=True, stop=False,
                            )
                            nc.tensor.matmul(
                                out=pp[:, v, lo:hi],
                                lhsT=xt[:, u * TT:(u + 1) * TT],
                                rhs=w_sb[:, lo:hi],
                                start=False, stop=True,
                            )
                    # Stage this PSUM pair -> its half of the fp16 SBUF group
                    nc.scalar.copy(
                        stg[:, sub * G:(sub + 1) * G, :], pp[:, :, 0:960]
                    )

                qmk = stg
                vw = stg[:, :, OV:OV + NV].rearrange("p g (o q) -> p g o q", q=4)

                # km[g,k,d] = K0[g,k,d] * msum[g,d]   (fp16, 2x)
                km = wpool.tile([TT, GS, KV, D], bf16)
                nc.vector.tensor_mul(
                    km[:],
                    qmk[:, :, OK:OK + NK].rearrange("p g (k d) -> p g k d", k=KV),
                    qmk[:, :, OM:OM + NM].unsqueeze(2).broadcast_to([TT, GS, KV, D]),
                )

                # P[g,h,k,d] = Q[g,h,d] * km[g,k,d]   (fp16, 2x; per-u: 3 free dims max)
                P = wpool.tile([TT, GS, H, KV, D], bf16)
                for u in range(GS):
                    nc.vector.tensor_mul(
                        P[:, u],
                        qmk[:, u, OQ:OQ + NQ]
                        .rearrange("p (h d) -> p h d", h=H)
                        .unsqueeze(2)
                        .broadcast_to([TT, H, KV, D]),
                        km[:, u].unsqueeze(1).broadcast_to([TT, H, KV, D]),
                    )

                # attn[g,h,k] = sum_d P: fold tree (2x adds) then 1x reduce
                A1 = wpool.tile([TT, GS, 8, 32], bf16)
                Pf = P[:].rearrange("p g h k d -> p g (h k) d")
                nc.vector.tensor_add(A1[:], Pf[:, :, :, 0:32], Pf[:, :, :, 32:64])
                A2 = wpool.tile([TT, GS, 8, 16], bf16)
                nc.vector.tensor_add(A2[:], A1[:, :, :, 0:16], A1[:, :, :, 16:32])
                attn = wpool.tile([TT, GS, 8], f32)
                nc.vector.tensor_reduce(
                    attn[:], A2[:], axis=mybir.AxisListType.X,
                    op=mybir.AluOpType.add,
                )
                attnb = wpool.tile([TT, GS, 8], bf16)
                nc.vector.tensor_copy(attnb[:], attn[:])

                # YP[g,j,o,q] = attn[g,j,q] * VW[g,o,q]   (fp16, 2x; per-u)
                YP = wpool.tile([TT, GS, 2, DIM, 4], bf16)
                for u in range(GS):
                    nc.vector.tensor_mul(
                        YP[:, u],
                        attnb[:, u].rearrange("p (j q) -> p j q", j=2)
                        .unsqueeze(2).broadcast_to([TT, 2, DIM, 4]),
                        vw[:, u].unsqueeze(1).broadcast_to([TT, 2, DIM, 4]),
                    )

                # Y[g,j,o] = sum_q YP: pair-fold (2x) + final add (1x, fp32)
                # (g,j) merged into one AP dim to stay within 3 free dims
                F = wpool.tile([TT, GS, 2, DIM, 2], bf16)
                YPm = YP[:].rearrange("p g j o q -> p (g j) o q")
                nc.vector.tensor_add(
                    F[:].rearrange("p g j o q -> p (g j) o q"),
                    YPm[:, :, :, 0:2], YPm[:, :, :, 2:4],
                )
                Y = ypool.tile([TT, GS, 2 * DIM], f32)
                Fm = F[:].rearrange("p g j o q -> p (g j) o q")
                nc.vector.tensor_add(
                    Y[:].rearrange("p g (j o) -> p (g j) o", j=2),
                    Fm[:, :, :, 0], Fm[:, :, :, 1],
                )

                for u in range(GS):
                    nc.sync.dma_start(
                        y_d[(g * GS + u) * TT:(g * GS + u + 1) * TT, :],
                        Y[:, u, :],
                    )

    nc.compile()
    return nc


def kernel(x, Wq, bq, Wk, bk, Wv, bv, Wo):
    global _COMPILED
    from concourse.bass_utils import run_bass_kernel_spmd

    x = np.asarray(x, dtype=np.float32)
    W_all, b_all = _fold_weights(
        np.asarray(Wq, np.float32), np.asarray(bq, np.float32),
        np.asarray(Wk, np.float32), np.asarray(bk, np.float32),
        np.asarray(Wv, np.float32), np.asarray(bv, np.float32),
        np.asarray(Wo, np.float32),
    )

    if _COMPILED is None:
        _COMPILED = _build_program()
    nc = _COMPILED

    x2d = x.reshape(T, DIM)
    ones = np.ones((1, TT), dtype=np.float32)
    ball = b_all.reshape(1, NPROJ)
    in_maps = []
    for c in range(NCORES):
        shard = x2d[c * TPC:(c + 1) * TPC]          # [16384, 128]
        in_maps.append({
            "xT": np.ascontiguousarray(shard.T),     # [128, 16384]
            "wall": W_all,
            "ball": ball,
            "ones": ones,
        })

    res = run_bass_kernel_spmd(nc, in_maps, list(range(NCORES)))
    ys = [res.results[c]["y"] for c in range(NCORES)]
    Y = np.concatenate(ys, axis=0)                   # [131072, 256]
    return Y.reshape(B, 2 * S, DIM)



# revision 3
# speedup vs baseline: 1.1304x; 1.1304x over previous
"""Trainium2 Bass kernel for nn_Attention_45148696216373.

8-core data-parallel over tokens (B*S = 131072 -> 16384/core); x is
pre-transposed on the host to [128c, tokens] fp16 so channel sits on
SBUF partitions for the PE matmul.

v2 ("u-path"): instead of folding Wo into a 512-wide per-token VW
projection (which made the DVE combine 1024 MAC/token), contract the
kv axis first in 64-dim space (u[h] = attn[h,0] v0 + attn[h,1] v1,
512 MAC/token) and apply the *shared* Wo via the PE: per 128-token
tile, PE-transpose u -> u2T [c, t'] and matmul against Wo^T. Work is
spread over four engines:

  PE   : 576-wide proj [Q(256)|msum(64)|K0(128)|Vi(128)] + bias rows,
         u transposes (fp16), y = u2 @ Wo^T matmuls
  ACT  : PSUM->SBUF fp16 staging (proj, u2T) + y f32 staging
  DVE  : km = K0*msum, P = q*km, d-fold chain -> attn, C = attn*v
  Pool : k-fold u = C[...,0] + C[...,1] (stride-2 op, 1x on DVE anyway)
  DMA  : x fp16 in (4MB), y f32 out (16MB)

V is staged (d,k)-interleaved so the C-mul runs at DVE 2x with k as
the innermost (step-1) axis of both broadcast operands. All scalar
constants (1/sqrt(D), 1/H) fold into the V columns on the host.

Math (per token t, all ops independent across tokens):
  q_st = x @ Wq^T + bq   -> [D,H] raw-reshaped to [H,D]  (index scramble)
  k_st, v_st similarly -> [KV,D]
  msum = sum_h q[h,:]
  km   = k0 * msum
  attn[h,k] = sum_d q[h,d]*km[k,d]          (scales folded into v')
  u[h,:]  = sum_k attn[h,k]*v'[k,:]         (v' = v/32)
  y[2t+j] = u[2j:2j+2].flat @ Wo^T
"""

import os

# The Bass SPMD path needs the axon trn2 PJRT backend; a cpu pin (e.g. from a
# reference-only harness env) would hide the 8 NeuronCores from jax.devices().
if os.environ.get("JAX_PLATFORMS", "").strip().lower() == "cpu":
    os.environ.pop("JAX_PLATFORMS")

import numpy as np

B, S, DIM = 16, 8192, 128
H, KV, D = 4, 2, 64
T = B * S                 # 131072 tokens
NCORES = 8
TPC = T // NCORES         # 16384 tokens per core
TT = 128                  # tokens per tile (partition dim)
NT = TPC // TT            # 128 tiles per core
GS = 16                   # tiles per group (DVE/ACT op batching)
NG = NT // GS             # 8 groups

NQ = H * D                # 256
NM = D                    # 64
NK = KV * D               # 128
NV = KV * D               # 128 (V block, (d,k)-interleaved cols)
NPROJ = NQ + NM + NK + NV  # 576
OQ, OM, OK, OV = 0, NQ, NQ + NM, NQ + NM + NK

_COMPILED = None


def _fold_weights(Wq, bq, Wk, bk, Wv, bv, Wo):
    """Build W_all [128, 576], bias_all [576], WoT [128, 128] (fp32)."""
    j = np.arange(NQ)
    Wq_p = Wq[j % H, j // H, :]            # [256, 128] col f=(h*64+d)
    bq_p = bq[j % H, j // H]               # [256]
    jk = np.arange(NK)
    Wk_p = Wk[jk % KV, jk // KV, :]        # [128, 128] col f=(k*64+d)
    bk_p = bk[jk % KV, jk // KV]
    Wv_p = Wv[jk % KV, jk // KV, :]        # [128, 128]
    bv_p = bv[jk % KV, jk // KV]

    # msum block: col d = sum_h Wq_p[h*64+d]
    Wm = Wq_p.reshape(H, D, DIM).sum(axis=0)     # [64, 128]
    bm = bq_p.reshape(H, D).sum(axis=0)          # [64]

    # V block, (d,k)-interleaved: col d*2+k = Wv_p[k*64+d] / 32
    # (1/32 = the 1/sqrt(D) attention scale times the 1/H of the q-mean)
    scale = 1.0 / 32.0
    Wv_i = (Wv_p.reshape(KV, D, DIM).transpose(1, 0, 2) * scale).reshape(NV, DIM)
    bv_i = (bv_p.reshape(KV, D).T * scale).reshape(NV)

    W_all = np.concatenate([Wq_p, Wm, Wk_p, Wv_i], axis=0)   # [576, 128]
    b_all = np.concatenate([bq_p, bm, bk_p, bv_i])           # [576]
    return W_all.T.copy(), b_all, Wo.T.copy()


def _numpy_forward(x2d, W_all, b_all, WoT):
    """Host re-implementation of the device math (for validation)."""
    proj = x2d @ W_all + b_all                         # [t, 576]
    Q = proj[:, OQ:OQ + NQ].reshape(-1, H, D)
    msum = proj[:, OM:OM + NM]
    K0 = proj[:, OK:OK + NK].reshape(-1, KV, D)
    Vi = proj[:, OV:OV + NV].reshape(-1, D, KV)        # [t, d, k]
    km = K0 * msum[:, None, :]
    attn = np.einsum("thd,tkd->thk", Q, km)            # [t, 4, 2]
    u = np.einsum("thk,tdk->thd", attn, Vi)            # [t, 4, 64]
    u2 = u.reshape(-1, 2, DIM)                         # [t, j, c]
    y = np.einsum("tjc,co->tjo", u2, WoT)              # [t, j, o]
    return y.reshape(-1, 2 * DIM)                      # [t, 256]


def _build_program():
    import concourse.bass as bass
    import concourse.tile as tile
    from concourse import bacc, mybir

    f32 = mybir.dt.float32
    f16 = mybir.dt.float16

    nc = bacc.Bacc(
        "TRN2",
        target_bir_lowering=False,
        debug=False,
        enable_asserts=False,
        num_devices=NCORES,
    )

    xT_d = nc.dram_tensor("xT", [DIM, TPC], f16, kind="ExternalInput").ap()
    w_d = nc.dram_tensor("wall", [DIM, NPROJ], f16, kind="ExternalInput").ap()
    b_d = nc.dram_tensor("ball", [1, NPROJ], f16, kind="ExternalInput").ap()
    one_d = nc.dram_tensor("ones", [1, TT], f16, kind="ExternalInput").ap()
    id_d = nc.dram_tensor("ident", [DIM, DIM], f16, kind="ExternalInput").ap()
    wo_d = nc.dram_tensor("woT", [DIM, DIM], f16, kind="ExternalInput").ap()
    y_d = nc.dram_tensor("y", [TPC, 2 * DIM], f32, kind="ExternalOutput").ap()

    with tile.TileContext(nc) as tc:
        with (
            tc.tile_pool(name="const", bufs=1) as cpool,
            tc.tile_pool(name="xin", bufs=2) as xpool,
            tc.tile_pool(name="pp", bufs=2, space="PSUM") as ppool,
            tc.tile_pool(name="utp", bufs=2, space="PSUM") as utppool,
            tc.tile_pool(name="yp", bufs=2, space="PSUM") as yppool,
            tc.tile_pool(name="stg", bufs=2) as spool,
            tc.tile_pool(name="work", bufs=2) as wpool,
            tc.tile_pool(name="u", bufs=2) as upool,
            tc.tile_pool(name="uts", bufs=2) as utspool,
            tc.tile_pool(name="ys", bufs=2) as yspool,
        ):
            w_sb = cpool.tile([DIM, NPROJ], f16)
            nc.sync.dma_start(w_sb[:], w_d[:, :])
            b_sb = cpool.tile([1, NPROJ], f16)
            nc.sync.dma_start(b_sb[:], b_d[:, :])
            one_sb = cpool.tile([1, TT], f16)
            nc.sync.dma_start(one_sb[:], one_d[:, :])
            id_sb = cpool.tile([DIM, DIM], f16)
            nc.sync.dma_start(id_sb[:], id_d[:, :])
            wo_sb = cpool.tile([DIM, DIM], f16)
            nc.sync.dma_start(wo_sb[:], wo_d[:, :])

            for g in range(NG):
                xt = xpool.tile([DIM, GS * TT], f16)
                nc.sync.dma_start(xt[:], xT_d[:, g * GS * TT:(g + 1) * GS * TT])

                stg = spool.tile([TT, GS, NPROJ], f16)
                for i in range(GS):
                    pp = ppool.tile([TT, NPROJ], f32, name=f"pp{i % 2}", tag="pp")
                    xi = xt[:, i * TT:(i + 1) * TT]
                    for lo, hi in ((0, 512), (512, NPROJ)):
                        nc.tensor.matmul(
                            out=pp[:, lo:hi], lhsT=one_sb[:, :],
                            rhs=b_sb[:, lo:hi], start=True, stop=False,
                        )
                    for lo, hi in ((0, 512), (512, NPROJ)):
                        nc.tensor.matmul(
                            out=pp[:, lo:hi], lhsT=xi,
                            rhs=w_sb[:, lo:hi], start=False, stop=True,
                        )
                    nc.scalar.copy(stg[:, i, :], pp[:, :])

                Q = stg[:, :, OQ:OQ + NQ].rearrange("p g (h d) -> p g h d", h=H)
                m = stg[:, :, OM:OM + NM]
                K0 = stg[:, :, OK:OK + NK].rearrange("p g (k d) -> p g k d", k=KV)
                Vi = stg[:, :, OV:OV + NV].rearrange("p g (d k) -> p g d k", k=KV)

                # km[g,k,d] = K0[g,k,d] * msum[g,d]   (fp16, 2x)
                km = wpool.tile([TT, GS, KV, D], f16)
                nc.vector.tensor_mul(
                    km[:], K0,
                    m.unsqueeze(2).broadcast_to([TT, GS, KV, D]),
                )

                # P[g,h,k,d] = Q[g,h,d] * km[g,k,d]  (per-h: <=3 free dims)
                P = wpool.tile([TT, GS, H, KV, D], f16)
                for h in range(H):
                    nc.vector.tensor_mul(
                        P[:, :, h],
                        Q[:, :, h].unsqueeze(2).broadcast_to([TT, GS, KV, D]),
                        km[:],
                    )

                # attn[g,h,k] = sum_d P : 2x fold tree over d
                Pf = P[:].rearrange("p g h k d -> p (g h k) d")
                A1 = wpool.tile([TT, GS * 8, 32], f16)
                nc.vector.tensor_add(A1[:], Pf[:, :, 0:32], Pf[:, :, 32:64])
                A2 = wpool.tile([TT, GS * 8, 16], f16)
                nc.vector.tensor_add(A2[:], A1[:, :, 0:16], A1[:, :, 16:32])
                A3 = wpool.tile([TT, GS * 8, 8], f16)
                nc.vector.tensor_add(A3[:], A2[:, :, 0:8], A2[:, :, 8:16])
                A4 = wpool.tile([TT, GS * 8, 4], f16)
                nc.vector.tensor_add(A4[:], A3[:, :, 0:4], A3[:, :, 4:8])
                A5 = wpool.tile([TT, GS * 8, 2], f16)
                nc.vector.tensor_add(A5[:], A4[:, :, 0:2], A4[:, :, 2:4])
                attn = wpool.tile([TT, GS, H, KV], f16)
                nc.vector.tensor_add(
                    attn[:].rearrange("p g h k -> p (g h k)"),
                    A5[:, :, 0], A5[:, :, 1],
                )

                # C[g,h,d,k] = attn[g,h,k] * Vi[g,d,k]  (k innermost: 2x)
                C = wpool.tile([TT, GS, H, D, KV], f16)
                for h in range(H):
                    nc.vector.tensor_mul(
                        C[:, :, h],
                        attn[:, :, h].unsqueeze(2).broadcast_to([TT, GS, D, KV]),
                        Vi,
                    )

                # u[g,h,d] = C[...,0] + C[...,1]  (stride-2: flat-rate GPSIMD)
                u = upool.tile([TT, GS, 2 * DIM], f16)
                uv = u[:].rearrange("p g (h d) -> p (g h) d", h=H)
                Cf = C[:].rearrange("p g h d k -> p (g h) d k")
                nc.gpsimd.tensor_add(uv, Cf[:, :, :, 0], Cf[:, :, :, 1])

                # Per 4 tiles: transpose u blocks, stage, y = u2 @ Wo^T
                for q in range(GS // 4):
                    utp = utppool.tile([DIM, 8, TT], f16, name="utp", tag="utp")
                    for ii in range(4):
                        i = q * 4 + ii
                        for j in range(2):
                            nc.tensor.transpose(
                                utp[:, ii * 2 + j, :],
                                u[:, i, j * DIM:(j + 1) * DIM],
                                id_sb[:, :],
                            )
                    uts = utspool.tile([DIM, 8, TT], f16)
                    nc.scalar.copy(uts[:], utp[:])
                    for half in range(2):
                        yp = yppool.tile([TT, 2, 2, DIM], f32, name="yp", tag="yp")
                        for i2 in range(2):
                            for j in range(2):
                                nc.tensor.matmul(
                                    out=yp[:, i2, j, :],
                                    lhsT=uts[:, (half * 2 + i2) * 2 + j, :],
                                    rhs=wo_sb[:, :],
                                    start=True, stop=True,
                                )
                        ys = yspool.tile([TT, 2, 2, DIM], f32)
                        nc.scalar.copy(ys[:], yp[:])
                        base = (g * GS + q * 4 + half * 2) * TT
                        dst = y_d[base:base + 2 * TT, :].rearrange(
                            "(i2 t) (j o) -> t i2 j o", i2=2, j=2
                        )
                        nc.sync.dma_start(dst, ys[:])

    nc.compile()
    return nc


def kernel(x, Wq, bq, Wk, bk, Wv, bv, Wo):
    global _COMPILED
    from concourse.bass_utils import run_bass_kernel_spmd

    x = np.asarray(x, dtype=np.float32)
    W_all, b_all, WoT = _fold_weights(
        np.asarray(Wq, np.float32), np.asarray(bq, np.float32),
        np.asarray(Wk, np.float32), np.asarray(bk, np.float32),
        np.asarray(Wv, np.float32), np.asarray(bv, np.float32),
        np.asarray(Wo, np.float32),
    )

    if _COMPILED is None:
        _COMPILED = _build_program()
    nc = _COMPILED

    x2d = x.reshape(T, DIM)
    ones = np.ones((1, TT), dtype=np.float16)
    ident = np.eye(DIM, dtype=np.float16)
    wall = W_all.astype(np.float16)
    ball = b_all.reshape(1, NPROJ).astype(np.float16)
    woT = WoT.astype(np.float16)
    in_maps = []
    for c in range(NCORES):
        shard = x2d[c * TPC:(c + 1) * TPC]          # [16384, 128]
        in_maps.append({
            "xT": np.ascontiguousarray(shard.T).astype(np.float16),
            "wall": wall,
            "ball": ball,
            "ones": ones,
            "ident": ident,
            "woT": woT,
        })

    res = run_bass_kernel_spmd(nc, in_maps, list(range(NCORES)))
    ys = [res.results[c]["y"] for c in range(NCORES)]
    Y = np.concatenate(ys, axis=0)                   # [131072, 256]
    return Y.reshape(B, 2 * S, DIM)


# revision 6
# speedup vs baseline: 1.1469x; 1.0146x over previous
"""Trainium2 Bass kernel for nn_Attention_45148696216373.

8-core data-parallel over tokens (B*S = 131072 -> 16384/core); x is
pre-transposed on the host to [128c, tokens] fp16 so channel sits on
SBUF partitions for the PE matmul.

v2 ("u-path"): instead of folding Wo into a 512-wide per-token VW
projection (which made the DVE combine 1024 MAC/token), contract the
kv axis first in 64-dim space (u[h] = attn[h,0] v0 + attn[h,1] v1,
512 MAC/token) and apply the *shared* Wo via the PE: per 128-token
tile, PE-transpose u -> u2T [c, t'] and matmul against Wo^T. Work is
spread over four engines:

  PE   : 576-wide proj [Q(256)|msum(64)|K0(128)|Vi(128)] + bias rows,
         u transposes (fp16), y = u2 @ Wo^T matmuls
  ACT  : PSUM->SBUF fp16 staging (proj, u2T) + y f32 staging
  DVE  : km = K0*msum, P = q*km, d-fold chain -> attn, C = attn*v
  Pool : k-fold u = C[...,0] + C[...,1] (stride-2 op, 1x on DVE anyway)
  DMA  : x fp16 in (4MB), y f32 out (16MB)

V is staged (d,k)-interleaved so the C-mul runs at DVE 2x with k as
the innermost (step-1) axis of both broadcast operands. All scalar
constants (1/sqrt(D), 1/H) fold into the V columns on the host.

Math (per token t, all ops independent across tokens):
  q_st = x @ Wq^T + bq   -> [D,H] raw-reshaped to [H,D]  (index scramble)
  k_st, v_st similarly -> [KV,D]
  msum = sum_h q[h,:]
  km   = k0 * msum
  attn[h,k] = sum_d q[h,d]*km[k,d]          (scales folded into v')
  u[h,:]  = sum_k attn[h,k]*v'[k,:]         (v' = v/32)
  y[2t+j] = u[2j:2j+2].flat @ Wo^T
"""

import os

# The Bass SPMD path needs the axon trn2 PJRT backend; a cpu pin (e.g. from a
# reference-only harness env) would hide the 8 NeuronCores from jax.devices().
if os.environ.get("JAX_PLATFORMS", "").strip().lower() == "cpu":
    os.environ.pop("JAX_PLATFORMS")

import numpy as np

B, S, DIM = 16, 8192, 128
H, KV, D = 4, 2, 64
T = B * S                 # 131072 tokens
NCORES = 8
TPC = T // NCORES         # 16384 tokens per core
TT = 128                  # tokens per tile (partition dim)
NT = TPC // TT            # 128 tiles per core
GS = 16                   # tiles per group (DVE/ACT op batching)
NG = NT // GS             # 8 groups

NQ = H * D                # 256
NM = D                    # 64
NK = KV * D               # 128
NV = KV * D               # 128 (V block, (d,k)-interleaved cols)
NPROJ = NQ + NM + NK + NV  # 576
OQ, OM, OK, OV = 0, NQ, NQ + NM, NQ + NM + NK

_COMPILED = None


def _fold_weights(Wq, bq, Wk, bk, Wv, bv, Wo):
    """Build W_all [128, 576], bias_all [576], WoT [128, 128] (fp32)."""
    j = np.arange(NQ)
    Wq_p = Wq[j % H, j // H, :]            # [256, 128] col f=(h*64+d)
    bq_p = bq[j % H, j // H]               # [256]
    jk = np.arange(NK)
    Wk_p = Wk[jk % KV, jk // KV, :]        # [128, 128] col f=(k*64+d)
    bk_p = bk[jk % KV, jk // KV]
    Wv_p = Wv[jk % KV, jk // KV, :]        # [128, 128]
    bv_p = bv[jk % KV, jk // KV]

    # msum block: col d = sum_h Wq_p[h*64+d]
    Wm = Wq_p.reshape(H, D, DIM).sum(axis=0)     # [64, 128]
    bm = bq_p.reshape(H, D).sum(axis=0)          # [64]

    # V block, (d,k)-interleaved: col d*2+k = Wv_p[k*64+d] / 32
    # (1/32 = the 1/sqrt(D) attention scale times the 1/H of the q-mean)
    scale = 1.0 / 32.0
    Wv_i = (Wv_p.reshape(KV, D, DIM).transpose(1, 0, 2) * scale).reshape(NV, DIM)
    bv_i = (bv_p.reshape(KV, D).T * scale).reshape(NV)

    W_all = np.concatenate([Wq_p, Wm, Wk_p, Wv_i], axis=0)   # [576, 128]
    b_all = np.concatenate([bq_p, bm, bk_p, bv_i])           # [576]
    return W_all.T.copy(), b_all, Wo.T.copy()


def _numpy_forward(x2d, W_all, b_all, WoT):
    """Host re-implementation of the device math (for validation)."""
    proj = x2d @ W_all + b_all                         # [t, 576]
    Q = proj[:, OQ:OQ + NQ].reshape(-1, H, D)
    msum = proj[:, OM:OM + NM]
    K0 = proj[:, OK:OK + NK].reshape(-1, KV, D)
    Vi = proj[:, OV:OV + NV].reshape(-1, D, KV)        # [t, d, k]
    km = K0 * msum[:, None, :]
    attn = np.einsum("thd,tkd->thk", Q, km)            # [t, 4, 2]
    u = np.einsum("thk,tdk->thd", attn, Vi)            # [t, 4, 64]
    u2 = u.reshape(-1, 2, DIM)                         # [t, j, c]
    y = np.einsum("tjc,co->tjo", u2, WoT)              # [t, j, o]
    return y.reshape(-1, 2 * DIM)                      # [t, 256]


def _build_program():
    import concourse.bass as bass
    import concourse.tile as tile
    from concourse import bacc, mybir

    f32 = mybir.dt.float32
    f16 = mybir.dt.float16

    nc = bacc.Bacc(
        "TRN2",
        target_bir_lowering=False,
        debug=False,
        enable_asserts=False,
        num_devices=NCORES,
    )

    xT_d = nc.dram_tensor("xT", [DIM, TPC], f16, kind="ExternalInput").ap()
    w_d = nc.dram_tensor("wall", [DIM, NPROJ], f16, kind="ExternalInput").ap()
    b_d = nc.dram_tensor("ball", [1, NPROJ], f16, kind="ExternalInput").ap()
    one_d = nc.dram_tensor("ones", [1, TT], f16, kind="ExternalInput").ap()
    id_d = nc.dram_tensor("ident", [DIM, DIM], f16, kind="ExternalInput").ap()
    wo_d = nc.dram_tensor("woT", [DIM, DIM], f16, kind="ExternalInput").ap()
    y_d = nc.dram_tensor("y", [TPC, 2 * DIM], f32, kind="ExternalOutput").ap()

    with tile.TileContext(nc) as tc:
        with (
            tc.tile_pool(name="const", bufs=1) as cpool,
            tc.tile_pool(name="xin", bufs=2) as xpool,
            tc.tile_pool(name="pp", bufs=2, space="PSUM") as ppool,
            tc.tile_pool(name="utp", bufs=2, space="PSUM") as utppool,
            tc.tile_pool(name="yp", bufs=2, space="PSUM") as yppool,
            tc.tile_pool(name="stg", bufs=2) as spool,
            tc.tile_pool(name="work", bufs=2) as wpool,
            tc.tile_pool(name="u", bufs=2) as upool,
            tc.tile_pool(name="uts", bufs=2) as utspool,
            tc.tile_pool(name="ys", bufs=2) as yspool,
        ):
            w_sb = cpool.tile([DIM, NPROJ], f16)
            nc.sync.dma_start(w_sb[:], w_d[:, :])
            b_sb = cpool.tile([1, NPROJ], f16)
            nc.sync.dma_start(b_sb[:], b_d[:, :])
            one_sb = cpool.tile([1, TT], f16)
            nc.sync.dma_start(one_sb[:], one_d[:, :])
            id_sb = cpool.tile([DIM, DIM], f16)
            nc.sync.dma_start(id_sb[:], id_d[:, :])
            wo_sb = cpool.tile([DIM, DIM], f16)
            nc.sync.dma_start(wo_sb[:], wo_d[:, :])

            def phase_a(g):
                """Group g: x DMA, projection matmuls, PSUM->SBUF staging."""
                xt = xpool.tile([DIM, GS * TT], f16)
                nc.sync.dma_start(xt[:], xT_d[:, g * GS * TT:(g + 1) * GS * TT])

                stg = spool.tile([TT, GS, NPROJ], f16)
                for i in range(GS):
                    pp = ppool.tile([TT, NPROJ], f32, name=f"pp{i % 2}", tag="pp")
                    xi = xt[:, i * TT:(i + 1) * TT]
                    for lo, hi in ((0, 512), (512, NPROJ)):
                        nc.tensor.matmul(
                            out=pp[:, lo:hi], lhsT=one_sb[:, :],
                            rhs=b_sb[:, lo:hi], start=True, stop=False,
                        )
                    for lo, hi in ((0, 512), (512, NPROJ)):
                        nc.tensor.matmul(
                            out=pp[:, lo:hi], lhsT=xi,
                            rhs=w_sb[:, lo:hi], start=False, stop=True,
                        )
                    nc.scalar.copy(stg[:, i, :], pp[:, :])
                return stg

            def phase_b(g, stg):
                """Group g: attention math, transposes, Wo matmul, y out."""
                Q = stg[:, :, OQ:OQ + NQ].rearrange("p g (h d) -> p g h d", h=H)
                m = stg[:, :, OM:OM + NM]
                K0 = stg[:, :, OK:OK + NK].rearrange("p g (k d) -> p g k d", k=KV)
                Vi = stg[:, :, OV:OV + NV].rearrange("p g (d k) -> p g d k", k=KV)

                # km[g,k,d] = K0[g,k,d] * msum[g,d]   (fp16, 2x)
                km = wpool.tile([TT, GS, KV, D], f16)
                nc.vector.tensor_mul(
                    km[:], K0,
                    m.unsqueeze(2).broadcast_to([TT, GS, KV, D]),
                )

                # P[g,h,k,d] = Q[g,h,d] * km[g,k,d]  (per-h: <=3 free dims)
                P = wpool.tile([TT, GS, H, KV, D], f16)
                for h in range(H):
                    nc.vector.tensor_mul(
                        P[:, :, h],
                        Q[:, :, h].unsqueeze(2).broadcast_to([TT, GS, KV, D]),
                        km[:],
                    )

                # attn[g,h,k] = sum_d P : 2x fold tree over d
                Pf = P[:].rearrange("p g h k d -> p (g h k) d")
                A1 = wpool.tile([TT, GS * 8, 32], f16)
                nc.vector.tensor_add(A1[:], Pf[:, :, 0:32], Pf[:, :, 32:64])
                A2 = wpool.tile([TT, GS * 8, 16], f16)
                nc.vector.tensor_add(A2[:], A1[:, :, 0:16], A1[:, :, 16:32])
                A3 = wpool.tile([TT, GS * 8, 8], f16)
                nc.vector.tensor_add(A3[:], A2[:, :, 0:8], A2[:, :, 8:16])
                A4 = wpool.tile([TT, GS * 8, 4], f16)
                nc.vector.tensor_add(A4[:], A3[:, :, 0:4], A3[:, :, 4:8])
                A5 = wpool.tile([TT, GS * 8, 2], f16)
                nc.vector.tensor_add(A5[:], A4[:, :, 0:2], A4[:, :, 2:4])
                attn = wpool.tile([TT, GS, H, KV], f16)
                nc.vector.tensor_add(
                    attn[:].rearrange("p g h k -> p (g h k)"),
                    A5[:, :, 0], A5[:, :, 1],
                )

                # C[g,h,d,k] = attn[g,h,k] * Vi[g,d,k]  (k innermost: 2x)
                C = wpool.tile([TT, GS, H, D, KV], f16)
                for h in range(H):
                    nc.vector.tensor_mul(
                        C[:, :, h],
                        attn[:, :, h].unsqueeze(2).broadcast_to([TT, GS, D, KV]),
                        Vi,
                    )

                # u[g,h,d] = C[...,0] + C[...,1]  (stride-2: flat-rate GPSIMD)
                u = upool.tile([TT, GS, 2 * DIM], f16)
                uv = u[:].rearrange("p g (h d) -> p (g h) d", h=H)
                Cf = C[:].rearrange("p g h d k -> p (g h) d k")
                nc.gpsimd.tensor_add(uv, Cf[:, :, :, 0], Cf[:, :, :, 1])

                # Per 4 tiles: transpose u blocks, stage, y = u2 @ Wo^T
                for q in range(GS // 4):
                    utp = utppool.tile([DIM, 8, TT], f16, name="utp", tag="utp")
                    for ii in range(4):
                        i = q * 4 + ii
                        for j in range(2):
                            nc.tensor.transpose(
                                utp[:, ii * 2 + j, :],
                                u[:, i, j * DIM:(j + 1) * DIM],
                                id_sb[:, :],
                            )
                    uts = utspool.tile([DIM, 8, TT], f16)
                    nc.scalar.copy(uts[:], utp[:])
                    for half in range(2):
                        yp = yppool.tile([TT, 2, 2, DIM], f32, name="yp", tag="yp")
                        for i2 in range(2):
                            for j in range(2):
                                nc.tensor.matmul(
                                    out=yp[:, i2, j, :],
                                    lhsT=uts[:, (half * 2 + i2) * 2 + j, :],
                                    rhs=wo_sb[:, :],
                                    start=True, stop=True,
                                )
                        ys = yspool.tile([TT, 2, 2, DIM], f32)
                        nc.scalar.copy(ys[:], yp[:])
                        base = (g * GS + q * 4 + half * 2) * TT
                        dst = y_d[base:base + 2 * TT, :].rearrange(
                            "(i2 t) (j o) -> t i2 j o", i2=2, j=2
                        )
                        nc.sync.dma_start(dst, ys[:])

            # Software pipeline: issue group g+1's projection+staging before
            # group g's tail so in-order engine queues never head-of-line
            # block the next group's critical path.
            stg_next = phase_a(0)
            for g in range(NG):
                stg_cur = stg_next
                if g + 1 < NG:
                    stg_next = phase_a(g + 1)
                phase_b(g, stg_cur)

    nc.compile()
    return nc


def kernel(x, Wq, bq, Wk, bk, Wv, bv, Wo):
    global _COMPILED
    from concourse.bass_utils import run_bass_kernel_spmd

    x = np.asarray(x, dtype=np.float32)
    W_all, b_all, WoT = _fold_weights(
        np.asarray(Wq, np.float32), np.asarray(bq, np.float32),
        np.asarray(Wk, np.float32), np.asarray(bk, np.float32),
        np.asarray(Wv, np.float32), np.asarray(bv, np.float32),
        np.asarray(Wo, np.float32),
    )

    if _COMPILED is None:
        _COMPILED = _build_program()
    nc = _COMPILED

    x2d = x.reshape(T, DIM)
    ones = np.ones((1, TT), dtype=np.float16)
    ident = np.eye(DIM, dtype=np.float16)
    wall = W_all.astype(np.float16)
    ball = b_all.reshape(1, NPROJ).astype(np.float16)
    woT = WoT.astype(np.float16)
    in_maps = []
    for c in range(NCORES):
        shard = x2d[c * TPC:(c + 1) * TPC]          # [16384, 128]
        in_maps.append({
            "xT": np.ascontiguousarray(shard.T).astype(np.float16),
            "wall": wall,
            "ball": ball,
            "ones": ones,
            "ident": ident,
            "woT": woT,
        })

    res = run_bass_kernel_spmd(nc, in_maps, list(range(NCORES)))
    ys = [res.results[c]["y"] for c in range(NCORES)]
    Y = np.concatenate(ys, axis=0)                   # [131072, 256]
    return Y.reshape(B, 2 * S, DIM)


# revision 8
# speedup vs baseline: 1.1810x; 1.0297x over previous
"""Trainium2 Bass kernel for nn_Attention_45148696216373.

8-core data-parallel over tokens (B*S = 131072 -> 16384/core); x is
pre-transposed on the host to [128c, tokens] fp16 so channel sits on
SBUF partitions for the PE matmul.

v2 ("u-path"): instead of folding Wo into a 512-wide per-token VW
projection (which made the DVE combine 1024 MAC/token), contract the
kv axis first in 64-dim space (u[h] = attn[h,0] v0 + attn[h,1] v1,
512 MAC/token) and apply the *shared* Wo via the PE: per 128-token
tile, PE-transpose u -> u2T [c, t'] and matmul against Wo^T. Work is
spread over four engines:

  PE   : 576-wide proj [Q(256)|msum(64)|K0(128)|Vi(128)] + bias rows,
         u transposes (fp16), y = u2 @ Wo^T matmuls
  ACT  : PSUM->SBUF fp16 staging (proj, u2T) + y f32 staging
  DVE  : km = K0*msum, P = q*km, d-fold chain -> attn, C = attn*v
  Pool : k-fold u = C[...,0] + C[...,1] (stride-2 op, 1x on DVE anyway)
  DMA  : x fp16 in (4MB), y f32 out (16MB)

V is staged (d,k)-interleaved so the C-mul runs at DVE 2x with k as
the innermost (step-1) axis of both broadcast operands. All scalar
constants (1/sqrt(D), 1/H) fold into the V columns on the host.

Math (per token t, all ops independent across tokens):
  q_st = x @ Wq^T + bq   -> [D,H] raw-reshaped to [H,D]  (index scramble)
  k_st, v_st similarly -> [KV,D]
  msum = sum_h q[h,:]
  km   = k0 * msum
  attn[h,k] = sum_d q[h,d]*km[k,d]          (scales folded into v')
  u[h,:]  = sum_k attn[h,k]*v'[k,:]         (v' = v/32)
  y[2t+j] = u[2j:2j+2].flat @ Wo^T
"""

import os

# The Bass SPMD path needs the axon trn2 PJRT backend; a cpu pin (e.g. from a
# reference-only harness env) would hide the 8 NeuronCores from jax.devices().
if os.environ.get("JAX_PLATFORMS", "").strip().lower() == "cpu":
    os.environ.pop("JAX_PLATFORMS")

import numpy as np

B, S, DIM = 16, 8192, 128
H, KV, D = 4, 2, 64
T = B * S                 # 131072 tokens
NCORES = 8
TPC = T // NCORES         # 16384 tokens per core
TT = 128                  # tokens per tile (partition dim)
NT = TPC // TT            # 128 tiles per core
GS = 16                   # tiles per group (DVE/ACT op batching)
NG = NT // GS             # 8 groups

NQ = H * D                # 256
NM = D                    # 64
NK = KV * D               # 128
NV = KV * D               # 128 (V block, (d,k)-interleaved cols)
NPROJ = NQ + NM + NK + NV  # 576
OQ, OM, OK, OV = 0, NQ, NQ + NM, NQ + NM + NK

_COMPILED = None


def _fold_weights(Wq, bq, Wk, bk, Wv, bv, Wo):
    """Build W_all [128, 576], bias_all [576], WoT [128, 128] (fp32)."""
    j = np.arange(NQ)
    Wq_p = Wq[j % H, j // H, :]            # [256, 128] col f=(h*64+d)
    bq_p = bq[j % H, j // H]               # [256]
    jk = np.arange(NK)
    Wk_p = Wk[jk % KV, jk // KV, :]        # [128, 128] col f=(k*64+d)
    bk_p = bk[jk % KV, jk // KV]
    Wv_p = Wv[jk % KV, jk // KV, :]        # [128, 128]
    bv_p = bv[jk % KV, jk // KV]

    # msum block: col d = sum_h Wq_p[h*64+d]
    Wm = Wq_p.reshape(H, D, DIM).sum(axis=0)     # [64, 128]
    bm = bq_p.reshape(H, D).sum(axis=0)          # [64]

    # V block, (d,k)-interleaved: col d*2+k = Wv_p[k*64+d] / 32
    # (1/32 = the 1/sqrt(D) attention scale times the 1/H of the q-mean)
    scale = 1.0 / 32.0
    Wv_i = (Wv_p.reshape(KV, D, DIM).transpose(1, 0, 2) * scale).reshape(NV, DIM)
    bv_i = (bv_p.reshape(KV, D).T * scale).reshape(NV)

    W_all = np.concatenate([Wq_p, Wm, Wk_p, Wv_i], axis=0)   # [576, 128]
    b_all = np.concatenate([bq_p, bm, bk_p, bv_i])           # [576]
    return W_all.T.copy(), b_all, Wo.T.copy()


def _numpy_forward(x2d, W_all, b_all, WoT):
    """Host re-implementation of the device math (for validation)."""
    proj = x2d @ W_all + b_all                         # [t, 576]
    Q = proj[:, OQ:OQ + NQ].reshape(-1, H, D)
    msum = proj[:, OM:OM + NM]
    K0 = proj[:, OK:OK + NK].reshape(-1, KV, D)
    Vi = proj[:, OV:OV + NV].reshape(-1, D, KV)        # [t, d, k]
    km = K0 * msum[:, None, :]
    attn = np.einsum("thd,tkd->thk", Q, km)            # [t, 4, 2]
    u = np.einsum("thk,tdk->thd", attn, Vi)            # [t, 4, 64]
    u2 = u.reshape(-1, 2, DIM)                         # [t, j, c]
    y = np.einsum("tjc,co->tjo", u2, WoT)              # [t, j, o]
    return y.reshape(-1, 2 * DIM)                      # [t, 256]


def _build_program():
    import concourse.bass as bass
    import concourse.tile as tile
    from concourse import bacc, mybir

    f32 = mybir.dt.float32
    f16 = mybir.dt.float16

    nc = bacc.Bacc(
        "TRN2",
        target_bir_lowering=False,
        debug=False,
        enable_asserts=False,
        num_devices=NCORES,
    )

    xT_d = nc.dram_tensor("xT", [DIM, TPC], f16, kind="ExternalInput").ap()
    w_d = nc.dram_tensor("wall", [DIM, NPROJ], f16, kind="ExternalInput").ap()
    b_d = nc.dram_tensor("ball", [1, NPROJ], f16, kind="ExternalInput").ap()
    one_d = nc.dram_tensor("ones", [1, TT], f16, kind="ExternalInput").ap()
    id_d = nc.dram_tensor("ident", [DIM, DIM], f16, kind="ExternalInput").ap()
    wo_d = nc.dram_tensor("woT", [DIM, DIM], f16, kind="ExternalInput").ap()
    y_d = nc.dram_tensor("y", [TPC, 2 * DIM], f32, kind="ExternalOutput").ap()

    with tile.TileContext(nc) as tc:
        with (
            tc.tile_pool(name="const", bufs=1) as cpool,
            tc.tile_pool(name="xin", bufs=2) as xpool,
            tc.tile_pool(name="pp", bufs=2, space="PSUM") as ppool,
            tc.tile_pool(name="utp", bufs=2, space="PSUM") as utppool,
            tc.tile_pool(name="yp", bufs=2, space="PSUM") as yppool,
            tc.tile_pool(name="stg", bufs=2) as spool,
            tc.tile_pool(name="work", bufs=2) as wpool,
            tc.tile_pool(name="u", bufs=2) as upool,
            tc.tile_pool(name="uts", bufs=2) as utspool,
            tc.tile_pool(name="ys", bufs=2) as yspool,
        ):
            w_sb = cpool.tile([DIM, NPROJ], f16)
            nc.sync.dma_start(w_sb[:], w_d[:, :])
            b_sb = cpool.tile([1, NPROJ], f16)
            nc.sync.dma_start(b_sb[:], b_d[:, :])
            one_sb = cpool.tile([1, TT], f16)
            nc.sync.dma_start(one_sb[:], one_d[:, :])
            id_sb = cpool.tile([DIM, DIM], f16)
            nc.sync.dma_start(id_sb[:], id_d[:, :])
            wo_sb = cpool.tile([DIM, DIM], f16)
            nc.sync.dma_start(wo_sb[:], wo_d[:, :])

            def phase_a(g):
                """Group g: x DMA, projection matmuls, PSUM->SBUF staging."""
                xt = xpool.tile([DIM, GS * TT], f16)
                nc.sync.dma_start(xt[:], xT_d[:, g * GS * TT:(g + 1) * GS * TT])

                stg = spool.tile([TT, GS, NPROJ], f16)
                for i in range(GS):
                    pp = ppool.tile([TT, NPROJ], f32, name=f"pp{i % 2}", tag="pp")
                    xi = xt[:, i * TT:(i + 1) * TT]
                    for lo, hi in ((0, 512), (512, NPROJ)):
                        nc.tensor.matmul(
                            out=pp[:, lo:hi], lhsT=one_sb[:, :],
                            rhs=b_sb[:, lo:hi], start=True, stop=False,
                        )
                    for lo, hi in ((0, 512), (512, NPROJ)):
                        nc.tensor.matmul(
                            out=pp[:, lo:hi], lhsT=xi,
                            rhs=w_sb[:, lo:hi], start=False, stop=True,
                        )
                    nc.scalar.copy(stg[:, i, :], pp[:, :])
                return stg

            def phase_b(g, stg):
                """Group g: attention math, transposes, Wo matmul, y out."""
                Q = stg[:, :, OQ:OQ + NQ].rearrange("p g (h d) -> p g h d", h=H)
                m = stg[:, :, OM:OM + NM]
                K0 = stg[:, :, OK:OK + NK].rearrange("p g (k d) -> p g k d", k=KV)
                Vi = stg[:, :, OV:OV + NV].rearrange("p g (d k) -> p g d k", k=KV)

                # km[g,k,d] = K0[g,k,d] * msum[g,d]   (fp16, 2x)
                km = wpool.tile([TT, GS, KV, D], f16)
                nc.vector.tensor_mul(
                    km[:], K0,
                    m.unsqueeze(2).broadcast_to([TT, GS, KV, D]),
                )

                # P[g,h,k,d] = Q[g,h,d] * km[g,k,d]  (per-h: <=3 free dims)
                P = wpool.tile([TT, GS, H, KV, D], f16)
                for h in range(H):
                    nc.vector.tensor_mul(
                        P[:, :, h],
                        Q[:, :, h].unsqueeze(2).broadcast_to([TT, GS, KV, D]),
                        km[:],
                    )

                # attn[g,h,k] = sum_d P : 2x fold tree over d
                Pf = P[:].rearrange("p g h k d -> p (g h k) d")
                A1 = wpool.tile([TT, GS * 8, 32], f16)
                nc.vector.tensor_add(A1[:], Pf[:, :, 0:32], Pf[:, :, 32:64])
                A2 = wpool.tile([TT, GS * 8, 16], f16)
                nc.vector.tensor_add(A2[:], A1[:, :, 0:16], A1[:, :, 16:32])
                A3 = wpool.tile([TT, GS * 8, 8], f16)
                nc.vector.tensor_add(A3[:], A2[:, :, 0:8], A2[:, :, 8:16])
                A4 = wpool.tile([TT, GS * 8, 4], f16)
                nc.vector.tensor_add(A4[:], A3[:, :, 0:4], A3[:, :, 4:8])
                A5 = wpool.tile([TT, GS * 8, 2], f16)
                nc.vector.tensor_add(A5[:], A4[:, :, 0:2], A4[:, :, 2:4])
                attn = wpool.tile([TT, GS, H, KV], f16)
                nc.vector.tensor_add(
                    attn[:].rearrange("p g h k -> p (g h k)"),
                    A5[:, :, 0], A5[:, :, 1],
                )

                # C[g,h,d,k] = attn[g,h,k] * Vi[g,d,k]  (k innermost: 2x)
                C = wpool.tile([TT, GS, H, D, KV], f16)
                for h in range(H):
                    nc.vector.tensor_mul(
                        C[:, :, h],
                        attn[:, :, h].unsqueeze(2).broadcast_to([TT, GS, D, KV]),
                        Vi,
                    )

                # u[g,h,d] = C[...,0] + C[...,1]  (stride-2: flat-rate GPSIMD)
                u = upool.tile([TT, GS, 2 * DIM], f16)
                uv = u[:].rearrange("p g (h d) -> p (g h) d", h=H)
                Cf = C[:].rearrange("p g h d k -> p (g h) d k")
                nc.gpsimd.tensor_add(uv, Cf[:, :, :, 0], Cf[:, :, :, 1])

                # Per 4 tiles: transpose u blocks, stage, y = u2 @ Wo^T
                ys = yspool.tile([TT, GS, 2, DIM], f32)
                for q in range(GS // 4):
                    utp = utppool.tile([DIM, 8, TT], f16, name="utp", tag="utp")
                    for ii in range(4):
                        i = q * 4 + ii
                        for j in range(2):
                            nc.tensor.transpose(
                                utp[:, ii * 2 + j, :],
                                u[:, i, j * DIM:(j + 1) * DIM],
                                id_sb[:, :],
                            )
                    uts = utspool.tile([DIM, 8, TT], f16)
                    nc.scalar.copy(uts[:], utp[:])
                    for half in range(2):
                        yp = yppool.tile([TT, 2, 2, DIM], f32, name="yp", tag="yp")
                        for i2 in range(2):
                            for j in range(2):
                                nc.tensor.matmul(
                                    out=yp[:, i2, j, :],
                                    lhsT=uts[:, (half * 2 + i2) * 2 + j, :],
                                    rhs=wo_sb[:, :],
                                    start=True, stop=True,
                                )
                        i0 = q * 4 + half * 2
                        nc.scalar.copy(ys[:, i0:i0 + 2], yp[:])
                # One y DMA per group: SWDGE descriptor-gen cost on the SP
                # sequencer (~1-3us per dma_start) made per-pair DMAs the
                # critical path.
                gbase = g * GS * TT
                dst = y_d[gbase:gbase + GS * TT, :].rearrange(
                    "(i t) (j o) -> t i j o", i=GS, j=2
                )
                nc.sync.dma_start(dst, ys[:])

            # Software pipeline: issue group g+1's projection+staging before
            # group g's tail so in-order engine queues never head-of-line
            # block the next group's critical path.
            stg_next = phase_a(0)
            for g in range(NG):
                stg_cur = stg_next
                if g + 1 < NG:
                    stg_next = phase_a(g + 1)
                phase_b(g, stg_cur)

    nc.compile()
    return nc


def kernel(x, Wq, bq, Wk, bk, Wv, bv, Wo):
    global _COMPILED
    from concourse.bass_utils import run_bass_kernel_spmd

    x = np.asarray(x, dtype=np.float32)
    W_all, b_all, WoT = _fold_weights(
        np.asarray(Wq, np.float32), np.asarray(bq, np.float32),
        np.asarray(Wk, np.float32), np.asarray(bk, np.float32),
        np.asarray(Wv, np.float32), np.asarray(bv, np.float32),
        np.asarray(Wo, np.float32),
    )

    if _COMPILED is None:
        _COMPILED = _build_program()
    nc = _COMPILED

    x2d = x.reshape(T, DIM)
    ones = np.ones((1, TT), dtype=np.float16)
    ident = np.eye(DIM, dtype=np.float16)
    wall = W_all.astype(np.float16)
    ball = b_all.reshape(1, NPROJ).astype(np.float16)
    woT = WoT.astype(np.float16)
    in_maps = []
    for c in range(NCORES):
        shard = x2d[c * TPC:(c + 1) * TPC]          # [16384, 128]
        in_maps.append({
            "xT": np.ascontiguousarray(shard.T).astype(np.float16),
            "wall": wall,
            "ball": ball,
            "ones": ones,
            "ident": ident,
            "woT": woT,
        })

    res = run_bass_kernel_spmd(nc, in_maps, list(range(NCORES)))
    ys = [res.results[c]["y"] for c in range(NCORES)]
    Y = np.concatenate(ys, axis=0)                   # [131072, 256]
    return Y.reshape(B, 2 * S, DIM)


# revision 17
# speedup vs baseline: 1.2741x; 1.0789x over previous
"""Trainium2 Bass kernel for nn_Attention_45148696216373.

8-core data-parallel over tokens (B*S = 131072 -> 16384/core); x is
pre-transposed on the host to [128c, tokens] fp16 so channel sits on
SBUF partitions for the PE matmul.

v2 ("u-path"): instead of folding Wo into a 512-wide per-token VW
projection (which made the DVE combine 1024 MAC/token), contract the
kv axis first in 64-dim space (u[h] = attn[h,0] v0 + attn[h,1] v1,
512 MAC/token) and apply the *shared* Wo via the PE: per 128-token
tile, PE-transpose u -> u2T [c, t'] and matmul against Wo^T. Work is
spread over four engines:

  PE   : 576-wide proj [Q(256)|msum(64)|K0(128)|Vi(128)] + bias rows,
         u transposes (fp16), y = u2 @ Wo^T matmuls
  ACT  : PSUM->SBUF fp16 staging (proj, u2T) + y f32 staging
  DVE  : km = K0*msum, P = q*km, d-fold chain -> attn, C = attn*v
  Pool : k-fold u = C[...,0] + C[...,1] (stride-2 op, 1x on DVE anyway)
  DMA  : x fp16 in (4MB), y f32 out (16MB)

V is staged (d,k)-interleaved so the C-mul runs at DVE 2x with k as
the innermost (step-1) axis of both broadcast operands. All scalar
constants (1/sqrt(D), 1/H) fold into the V columns on the host.

Math (per token t, all ops independent across tokens):
  q_st = x @ Wq^T + bq   -> [D,H] raw-reshaped to [H,D]  (index scramble)
  k_st, v_st similarly -> [KV,D]
  msum = sum_h q[h,:]
  km   = k0 * msum
  attn[h,k] = sum_d q[h,d]*km[k,d]          (scales folded into v')
  u[h,:]  = sum_k attn[h,k]*v'[k,:]         (v' = v/32)
  y[2t+j] = u[2j:2j+2].flat @ Wo^T
"""

import os

# The Bass SPMD path needs the axon trn2 PJRT backend; a cpu pin (e.g. from a
# reference-only harness env) would hide the 8 NeuronCores from jax.devices().
if os.environ.get("JAX_PLATFORMS", "").strip().lower() == "cpu":
    os.environ.pop("JAX_PLATFORMS")

import numpy as np

B, S, DIM = 16, 8192, 128
H, KV, D = 4, 2, 64
T = B * S                 # 131072 tokens
NCORES = 8
TPC = T // NCORES         # 16384 tokens per core
TT = 128                  # tokens per tile (partition dim)
NT = TPC // TT            # 128 tiles per core
GS = 16                   # tiles per group (DVE/ACT op batching)
NG = NT // GS             # 8 groups

NQ = H * D                # 256
NM = D                    # 64
NK = KV * D               # 128
NV = KV * D               # 128 (V block, (d,k)-interleaved cols)
NPROJ = NQ + NM + NK + NV  # 576
OQ, OM, OK, OV = 0, NQ, NQ + NM, NQ + NM + NK

_COMPILED = None


def _fold_weights(Wq, bq, Wk, bk, Wv, bv, Wo):
    """Build W_all [128, 576], bias_all [576], WoT [128, 128] (fp32)."""
    j = np.arange(NQ)
    Wq_p = Wq[j % H, j // H, :]            # [256, 128] col f=(h*64+d)
    bq_p = bq[j % H, j // H]               # [256]
    jk = np.arange(NK)
    Wk_p = Wk[jk % KV, jk // KV, :]        # [128, 128] col f=(k*64+d)
    bk_p = bk[jk % KV, jk // KV]
    Wv_p = Wv[jk % KV, jk // KV, :]        # [128, 128]
    bv_p = bv[jk % KV, jk // KV]

    # msum block: col d = sum_h Wq_p[h*64+d]
    Wm = Wq_p.reshape(H, D, DIM).sum(axis=0)     # [64, 128]
    bm = bq_p.reshape(H, D).sum(axis=0)          # [64]

    # V block, (d,k)-interleaved: col d*2+k = Wv_p[k*64+d] / 32
    # (1/32 = the 1/sqrt(D) attention scale times the 1/H of the q-mean)
    scale = 1.0 / 32.0
    Wv_i = (Wv_p.reshape(KV, D, DIM).transpose(1, 0, 2) * scale).reshape(NV, DIM)
    bv_i = (bv_p.reshape(KV, D).T * scale).reshape(NV)

    W_all = np.concatenate([Wq_p, Wm, Wk_p, Wv_i], axis=0)   # [576, 128]
    b_all = np.concatenate([bq_p, bm, bk_p, bv_i])           # [576]
    return W_all.T.copy(), b_all, Wo.T.copy()


def _numpy_forward(x2d, W_all, b_all, WoT):
    """Host re-implementation of the device math (for validation)."""
    proj = x2d @ W_all + b_all                         # [t, 576]
    Q = proj[:, OQ:OQ + NQ].reshape(-1, H, D)
    msum = proj[:, OM:OM + NM]
    K0 = proj[:, OK:OK + NK].reshape(-1, KV, D)
    Vi = proj[:, OV:OV + NV].reshape(-1, D, KV)        # [t, d, k]
    km = K0 * msum[:, None, :]
    attn = np.einsum("thd,tkd->thk", Q, km)            # [t, 4, 2]
    u = np.einsum("thk,tdk->thd", attn, Vi)            # [t, 4, 64]
    u2 = u.reshape(-1, 2, DIM)                         # [t, j, c]
    y = np.einsum("tjc,co->tjo", u2, WoT)              # [t, j, o]
    return y.reshape(-1, 2 * DIM)                      # [t, 256]


def _build_program():
    import concourse.bass as bass
    import concourse.tile as tile
    from concourse import bacc, mybir

    f32 = mybir.dt.float32
    f16 = mybir.dt.float16

    nc = bacc.Bacc(
        "TRN2",
        target_bir_lowering=False,
        debug=False,
        enable_asserts=False,
        num_devices=NCORES,
    )

    xT_d = nc.dram_tensor("xT", [DIM, TPC], f16, kind="ExternalInput").ap()
    w_d = nc.dram_tensor("wall", [DIM, NPROJ], f16, kind="ExternalInput").ap()
    b_d = nc.dram_tensor("ball", [1, NPROJ], f16, kind="ExternalInput").ap()
    one_d = nc.dram_tensor("ones", [1, TT], f16, kind="ExternalInput").ap()
    wo_d = nc.dram_tensor("woT", [DIM, DIM], f16, kind="ExternalInput").ap()
    y_d = nc.dram_tensor("y", [TPC, 2 * DIM], f32, kind="ExternalOutput").ap()

    with tile.TileContext(nc) as tc:
        with (
            tc.tile_pool(name="const", bufs=1) as cpool,
            tc.tile_pool(name="xin", bufs=2) as xpool,
            tc.tile_pool(name="pp", bufs=2, space="PSUM") as ppool,
            tc.tile_pool(name="yp", bufs=2, space="PSUM") as yppool,
            tc.tile_pool(name="stg", bufs=2) as spool,
            # DVE-only intermediates: single-buffered (DVE is in-order, so
            # group g+1's writes can never race group g's reads)
            tc.tile_pool(name="work", bufs=1) as wpool,
            tc.tile_pool(name="cc", bufs=2) as cpool2,
            tc.tile_pool(name="u", bufs=2) as upool,
            tc.tile_pool(name="u2t", bufs=2) as u2tpool,
            tc.tile_pool(name="ys", bufs=2) as yspool,
        ):
            w_sb = cpool.tile([DIM, NPROJ], f16)
            nc.sync.dma_start(w_sb[:], w_d[:, :])
            b_sb = cpool.tile([1, NPROJ], f16)
            nc.sync.dma_start(b_sb[:], b_d[:, :])
            one_sb = cpool.tile([1, TT], f16)
            nc.sync.dma_start(one_sb[:], one_d[:, :])
            wo_sb = cpool.tile([DIM, DIM], f16)
            nc.sync.dma_start(wo_sb[:], wo_d[:, :])

            def phase_a(g):
                """Group g: x DMA, projection matmuls, PSUM->SBUF staging."""
                xt = xpool.tile([DIM, GS * TT], f16)
                nc.sync.dma_start(xt[:], xT_d[:, g * GS * TT:(g + 1) * GS * TT])

                stg = spool.tile([TT, GS, NPROJ], f16)
                for i in range(GS):
                    pp = ppool.tile([TT, NPROJ], f32, name=f"pp{i % 2}", tag="pp")
                    xi = xt[:, i * TT:(i + 1) * TT]
                    for lo, hi in ((0, 512), (512, NPROJ)):
                        nc.tensor.matmul(
                            out=pp[:, lo:hi], lhsT=one_sb[:, :],
                            rhs=b_sb[:, lo:hi], start=True, stop=False,
                        )
                        nc.tensor.matmul(
                            out=pp[:, lo:hi], lhsT=xi,
                            rhs=w_sb[:, lo:hi], start=False, stop=True,
                        )
                    nc.scalar.copy(stg[:, i, :], pp[:, :])
                return stg

            def phase_b(g, stg):
                """Group g: attention math, transposes, Wo matmul, y out."""
                Q = stg[:, :, OQ:OQ + NQ].rearrange("p g (h d) -> p g h d", h=H)
                m = stg[:, :, OM:OM + NM]
                K0 = stg[:, :, OK:OK + NK].rearrange("p g (k d) -> p g k d", k=KV)
                Vi = stg[:, :, OV:OV + NV].rearrange("p g (d k) -> p g d k", k=KV)

                # km[g,k,d] = K0[g,k,d] * msum[g,d]   (fp16, 2x)
                km = wpool.tile([TT, GS, KV, D], f16)
                nc.vector.tensor_mul(
                    km[:], K0,
                    m.unsqueeze(2).broadcast_to([TT, GS, KV, D]),
                )

                # P[g,h,k,d] = Q[g,h,d] * km[g,k,d]  (per-h: <=3 free dims)
                P = wpool.tile([TT, GS, H, KV, D], f16)
                for h in range(H):
                    nc.vector.tensor_mul(
                        P[:, :, h],
                        Q[:, :, h].unsqueeze(2).broadcast_to([TT, GS, KV, D]),
                        km[:],
                    )

                # attn[g,h,k] = sum_d P : 2x fold tree over d
                Pf = P[:].rearrange("p g h k d -> p (g h k) d")
                A1 = wpool.tile([TT, GS * 8, 32], f16)
                nc.vector.tensor_add(A1[:], Pf[:, :, 0:32], Pf[:, :, 32:64])
                A2 = wpool.tile([TT, GS * 8, 16], f16)
                nc.vector.tensor_add(A2[:], A1[:, :, 0:16], A1[:, :, 16:32])
                A3 = wpool.tile([TT, GS * 8, 8], f16)
                nc.vector.tensor_add(A3[:], A2[:, :, 0:8], A2[:, :, 8:16])
                A4 = wpool.tile([TT, GS * 8, 4], f16)
                nc.vector.tensor_add(A4[:], A3[:, :, 0:4], A3[:, :, 4:8])
                A5 = wpool.tile([TT, GS * 8, 2], f16)
                nc.vector.tensor_add(A5[:], A4[:, :, 0:2], A4[:, :, 2:4])
                attn = wpool.tile([TT, GS, H, KV], f16)
                nc.vector.tensor_add(
                    attn[:].rearrange("p g h k -> p (g h k)"),
                    A5[:, :, 0], A5[:, :, 1],
                )

                # C[g,h,d,k] = attn[g,h,k] * Vi[g,d,k]  (k innermost: 2x)
                C = cpool2.tile([TT, GS, H, D, KV], f16)
                for h in range(H):
                    nc.vector.tensor_mul(
                        C[:, :, h],
                        attn[:, :, h].unsqueeze(2).broadcast_to([TT, GS, D, KV]),
                        Vi,
                    )

                # u[g,h,d] = C[...,0] + C[...,1]  (stride-2: flat-rate GPSIMD)
                u = upool.tile([TT, GS, 2 * DIM], f16)
                uv = u[:].rearrange("p g (h d) -> p (g h) d", h=H)
                Cf = C[:].rearrange("p g h d k -> p (g h) d k")
                nc.gpsimd.tensor_add(uv, Cf[:, :, :, 0], Cf[:, :, :, 1])

                # Batched u transpose through the DMA xbar: one instruction
                # block-transposes all GS*2 [128,128] u blocks (no PE
                # transposes, no ACT staging of the transposed data).
                u2t = u2tpool.tile([DIM, GS * 2, TT], f16)
                nc.sync.dma_start_transpose(
                    u2t[:], u[:].rearrange("p g c -> p (g c)")
                )

                # y = u2 @ Wo^T per tile/j; stage f32 PSUM->SBUF per 4 tiles
                ys = yspool.tile([TT, GS, 2, DIM], f32)
                for q in range(GS // 4):
                    yp = yppool.tile([TT, 4, 2, DIM], f32, name="yp", tag="yp")
                    for ii in range(4):
                        for j in range(2):
                            nc.tensor.matmul(
                                out=yp[:, ii, j, :],
                                lhsT=u2t[:, (q * 4 + ii) * 2 + j, :],
                                rhs=wo_sb[:, :],
                                start=True, stop=True,
                            )
                    nc.scalar.copy(ys[:, q * 4:(q + 1) * 4], yp[:])
                # One y DMA per group: SWDGE descriptor-gen cost on the SP
                # sequencer (~1-3us per dma_start) made per-pair DMAs the
                # critical path.
                gbase = g * GS * TT
                dst = y_d[gbase:gbase + GS * TT, :].rearrange(
                    "(i t) (j o) -> t i j o", i=GS, j=2
                )
                nc.sync.dma_start(dst, ys[:])

            # Software pipeline: issue group g+1's projection+staging before
            # group g's tail so in-order engine queues never head-of-line
            # block the next group's critical path.
            stg_next = phase_a(0)
            for g in range(NG):
                stg_cur = stg_next
                if g + 1 < NG:
                    stg_next = phase_a(g + 1)
                phase_b(g, stg_cur)

    nc.compile()
    return nc


def kernel(x, Wq, bq, Wk, bk, Wv, bv, Wo):
    global _COMPILED
    from concourse.bass_utils import run_bass_kernel_spmd

    x = np.asarray(x, dtype=np.float32)
    W_all, b_all, WoT = _fold_weights(
        np.asarray(Wq, np.float32), np.asarray(bq, np.float32),
        np.asarray(Wk, np.float32), np.asarray(bk, np.float32),
        np.asarray(Wv, np.float32), np.asarray(bv, np.float32),
        np.asarray(Wo, np.float32),
    )

    if _COMPILED is None:
        _COMPILED = _build_program()
    nc = _COMPILED

    x2d = x.reshape(T, DIM)
    ones = np.ones((1, TT), dtype=np.float16)
    wall = W_all.astype(np.float16)
    ball = b_all.reshape(1, NPROJ).astype(np.float16)
    woT = WoT.astype(np.float16)
    in_maps = []
    for c in range(NCORES):
        shard = x2d[c * TPC:(c + 1) * TPC]          # [16384, 128]
        in_maps.append({
            "xT": np.ascontiguousarray(shard.T).astype(np.float16),
            "wall": wall,
            "ball": ball,
            "ones": ones,
            "woT": woT,
        })

    res = run_bass_kernel_spmd(nc, in_maps, list(range(NCORES)))
    ys = [res.results[c]["y"] for c in range(NCORES)]
    Y = np.concatenate(ys, axis=0)                   # [131072, 256]
    return Y.reshape(B, 2 * S, DIM)
